# revision 21
# baseline (speedup 1.0000x reference)
"""Trainium2 Bass kernel for nn_DecoderRNN (LSTM decoder + big vocab projection).

Strategy (8 NeuronCores, SPMD):
  - LSTM recurrence (B=32, T=64, H=512) replicated on every core (its cost is
    batch-independent); output projection fc tensor-parallel over vocab:
    core c writes logits[:, :, 1250c:1250(c+1)], host concatenates.
  - The input-projection term of the gates is precomputed as a parameter-only
    product on the host:  G_table = embed @ W_ih.T + (b_ih + b_hh)  [V, 4H].
    On device, step t's xs-contribution is a single indirect-DMA gather of
    G_table rows by caption index (t=0 rows come from the host-computed
    features @ W_ih.T + b).  This replaces the on-device xs @ W_ih.T matmuls,
    the embedding gather + transposes, and all bias matmuls.
  - Gate pre-activations for step t accumulate in a [32, 512] PSUM bank per
    gate chunk: a stacked-identity "inject" matmul deposits the gathered
    xs-term (emitted at the end of step t-1, so it streams during the tail),
    then two fp8 DoubleRow matmuls add h(t-1) @ W_hh.T and close the group.
  - Scaling: G_table is host-prescaled x1024; hsT8 = 16x h, W_hh = 64x ->
    gate PSUM = 1024x true; activations descale by 1/1024.
  - Gate chunk order in SBUF columns is [f | i | g | o]: sigmoid(f) (which
    heads the c critical path via f*c) completes first.  The f*c multiply
    runs on the otherwise-idle gpsimd engine; the rest of the tail is on
    Vector.  fc drains emit bf16 and the host upcasts.

PSUM budget (8 banks): 5 gate banks (4 live + staging for next step's
injects) + 2 fc banks + 1 transpose bank.

kernel(**inputs) takes FULL unsharded inputs, returns FULL [32, 64, 10000] f32.
"""

import sys

sys.path.insert(0, "/opt/trn_rl_repo")

import numpy as np

N_CORES = 8
B, T = 32, 64
E, H, V = 512, 512, 10000
G4 = 4 * H            # 2048
TB = T * B            # 2048
VSL = V // N_CORES    # 1250 vocab rows per core
VPAD = 1280           # padded so fc N-chunks are 512/512/256 (all >=256)

_PROGRAM = None


def _build_program():
    import concourse.bass as bass
    import concourse.tile as tile
    from concourse import bacc, mybir
    from concourse.masks import make_identity
    from contextlib import ExitStack

    f32 = mybir.dt.float32
    bf16 = mybir.dt.bfloat16
    f8e4 = mybir.dt.float8e4
    i32 = mybir.dt.int32
    AF = mybir.ActivationFunctionType
    DR = mybir.MatmulPerfMode.DoubleRow

    nc = bacc.Bacc(
        "TRN2",
        target_bir_lowering=False,
        debug=False,
        num_devices=N_CORES,
    )

    xp0 = nc.dram_tensor("xp0", [B, G4], bf16, kind="ExternalInput").ap()
    idx = nc.dram_tensor("idx", [TB], i32, kind="ExternalInput").ap()
    gtab = nc.dram_tensor("gtab", [V, G4], bf16, kind="ExternalInput").ap()
    whh8 = nc.dram_tensor("whh8", [H, G4], f8e4, kind="ExternalInput").ap()
    fcwT = nc.dram_tensor("fcwT", [H, VPAD], bf16, kind="ExternalInput").ap()
    fcb = nc.dram_tensor("fcb", [VPAD], bf16, kind="ExternalInput").ap()
    onesv = nc.dram_tensor("onesv", [128], bf16, kind="ExternalInput").ap()
    out = nc.dram_tensor("out", [B, T, VSL], bf16, kind="ExternalOutput").ap()
    # Output viewed as [t, b, v]: a 128-row t-major tb tile = 4 t planes.
    out_r = out.rearrange("b t v -> t b v")

    with tile.TileContext(nc) as tc, ExitStack() as ctx:
        # ---------------- persistent state ----------------
        state = ctx.enter_context(tc.tile_pool(name="state", bufs=1))
        # h.T history: block t holds h(t).T (written at the end of step t).
        # Layout [p, k, 32*t + b] = h(t)[b, 128*k + p]
        hsT = state.tile([128, 4, 32 * T], bf16, tag="hsT")
        # fp8 16x copy for the DoubleRow recurrence matmuls: pair layout
        # [p, P, i, .] = contraction row 128*(2P+i)+p.
        hsT8 = state.tile([128, 2, 2, 32 * T], f8e4, tag="hsT8")
        whh8_sb = state.tile([128, 2, 2, G4], f8e4, tag="whh8")
        fcwT_sb = state.tile([128, 4, VPAD], bf16, tag="fcwT")
        fcb_sb = state.tile([1, VPAD], bf16, tag="fcb")
        fcb128 = state.tile([128, VPAD], f32, tag="fcb128")
        c_sb = state.tile([B, H], bf16, tag="c")
        ident_b = state.tile([128, 128], bf16, tag="ident_b")
        ones = state.tile([1, 128], bf16, tag="ones")
        eye4 = state.tile([128, 32], bf16, tag="eye4")   # I32 stacked 4x
        idx_sb = state.tile([128, 16], i32, tag="idx")

        make_identity(nc, ident_b[:])
        nc.vector.memset(c_sb[:], 0.0)

        nc.sync.dma_start(fcb_sb[:], fcb[None, :])
        nc.sync.dma_start(ones[:], onesv[None, :])
        for qq in range(4):
            nc.sync.dma_start(eye4[32 * qq : 32 * (qq + 1), :], ident_b[0:32, 0:32])

        # ---------------- PSUM pools ----------------
        g_psum = ctx.enter_context(tc.tile_pool(name="g_ps", bufs=5, space="PSUM"))
        fc_psum = ctx.enter_context(tc.tile_pool(name="fc_ps", bufs=2, space="PSUM"))
        h_psum = ctx.enter_context(tc.tile_pool(name="h_ps", bufs=1, space="PSUM"))

        # gathered G_table rows (xs gate contribution), [tb-tile, 4H] ring
        xp_ring = ctx.enter_context(tc.tile_pool(name="xp_ring", bufs=3))
        xp_tiles = {}
        gate_tiles = {}

        def gather_xp(m):
            """xp tile m = G_table rows for tb 128m..128m+128 (gate space)."""
            xp_m = xp_ring.tile([128, G4], bf16, tag="xp_m")
            xp_tiles[m] = xp_m
            nc.gpsimd.indirect_dma_start(
                out=xp_m[:],
                out_offset=None,
                in_=gtab[:, :],
                in_offset=bass.IndirectOffsetOnAxis(
                    ap=idx_sb[:, m : m + 1], axis=0
                ),
            )
            if m == 0:
                # t=0 rows: host-computed features @ W_ih.T + b
                nc.gpsimd.dma_start(xp_m[0:32, :], xp0[:, :])

        def emit_inject(t):
            """Open step t's four chunk groups with the gathered xs term."""
            q = t % 4
            xp_m = xp_tiles[t // 4]
            for cch in range(4):
                sl = slice(512 * cch, 512 * (cch + 1))
                gt = g_psum.tile([B, 512], f32, tag="g")
                gate_tiles[(t, cch)] = gt
                nc.tensor.matmul(
                    gt[:],
                    lhsT=eye4[32 * q : 32 * (q + 1), :],
                    rhs=xp_m[32 * q : 32 * (q + 1), sl],
                    start=True,
                    stop=(t == 0),
                    tile_position=(32 * q, 0),
                    skip_group_check=True,
                )

        # ---------------- prologue ----------------
        with ExitStack() as pro:
            nc.sync.dma_start(idx_sb[:], idx.rearrange("(m p) -> p m", p=128))
            nc.sync.dma_start(
                whh8_sb[:], whh8.rearrange("(P i p) g -> p P i g", P=2, i=2)
            )
            nc.sync.dma_start(fcwT_sb[:], fcwT.rearrange("(k p) v -> p k v", p=128))
            # fcb128 = broadcast(fc_b) via rank-1 matmuls into the fc bank
            for c0, csz in ((0, 512), (512, 512), (1024, 256)):
                bp = fc_psum.tile([128, 512], f32, tag="fc")
                nc.tensor.matmul(bp[:, 0:csz], lhsT=ones[0:1, :],
                                 rhs=fcb_sb[0:1, c0 : c0 + csz], start=True, stop=True)
                nc.vector.tensor_copy(fcb128[:, c0 : c0 + csz], bp[:, 0:csz])

            gather_xp(0)
            gather_xp(1)
            emit_inject(0)

        # ---------------- main recurrence + interleaved fc ----------------
        work = ctx.enter_context(tc.tile_pool(name="work", bufs=3))
        lg_pool = ctx.enter_context(tc.tile_pool(name="lg", bufs=2))

        FC_CHUNKS = ((0, 512), (512, 512), (1024, 256))
        lg_tiles = {}

        def fc_chunk_mms(m, j):
            """PE part of fc chunk j for tb tile m (fills PE bubbles)."""
            if j == 0:
                lg_new = lg_pool.tile([128, VPAD], bf16, tag="lg")
                lg_tiles[m] = lg_new
            c0, csz = FC_CHUNKS[j]
            fps = fc_psum.tile([128, 512], f32, tag="fc")
            for k in range(4):
                nc.tensor.matmul(
                    fps[:, 0:csz],
                    lhsT=hsT[:, k, 128 * m : 128 * (m + 1)],
                    rhs=fcwT_sb[:, k, c0 : c0 + csz],
                    start=(k == 0),
                    stop=(k == 3),
                )
            return fps

        def fc_chunk_finish(m, j, fps):
            c0, csz = FC_CHUNKS[j]
            nc.vector.tensor_add(
                lg_tiles[m][:, c0 : c0 + csz], fps[:, 0:csz], fcb128[:, c0 : c0 + csz]
            )
            if j == 2:
                # DRAM side is [4 t, 32 b, 1250 v]; SBUF side [128, 1250]
                # pairs element-stream-wise (partition p = 32*t_local + b).
                nc.sync.dma_start(
                    out_r[4 * m : 4 * (m + 1), :, :], lg_tiles[m][:, 0:VSL]
                )

        # gate chunk order in SBUF columns (host permutes): 0=f 1=i 2=g 3=o
        for t in range(T):
            q = t % 4
            m = t // 4
            nl = work.tile([B, G4], bf16, tag="nl")

            # ---- close the chunk groups with h(t-1) @ W_hh.T; act ASAP ----
            # f-chunk runs in 256-col quarters so sigmoid(f) half 0 (which
            # heads the c critical path) starts after just two matmuls.
            for cch in range(4):
                gt = gate_tiles[(t, cch)]
                if t > 0:
                    nhalves = (0, 1) if cch == 0 else (None,)
                    for nh in nhalves:
                        ns = slice(0, 512) if nh is None else slice(256 * nh, 256 * (nh + 1))
                        for P in (0, 1):
                            nc.tensor.matmul(
                                gt[:, ns],
                                lhsT=hsT8[:, P, :, 32 * (t - 1) : 32 * t],
                                rhs=whh8_sb[:, P, :, 512 * cch + ns.start : 512 * cch + ns.stop],
                                start=False,
                                stop=(P == 1),
                                perf_mode=DR,
                                skip_group_check=True,
                            )
            # Scalar act queue order (f0, i, g, f1, o0, o1): f0 heads the
            # f*c chain; i and g feed sigmoid(i)*tanh(g) next; f1 (whose
            # consumer f*c-half1 runs on gpsimd, off the critical path) is
            # deferred behind g so the ig-product path isn't delayed.
            def act(cch, ah=None):
                g_tile = gate_tiles[(t, cch)]
                if ah is None:
                    nc.scalar.activation(
                        nl[:, 512 * cch : 512 * (cch + 1)], g_tile[:],
                        AF.Tanh if cch == 2 else AF.Sigmoid, scale=1.0 / 1024.0,
                    )
                else:
                    nc.scalar.activation(
                        nl[:, 512 * cch + 256 * ah : 512 * cch + 256 * (ah + 1)],
                        g_tile[:, 256 * ah : 256 * (ah + 1)],
                        AF.Sigmoid, scale=1.0 / 1024.0,
                    )

            act(0, 0)
            act(1)
            act(2)
            act(0, 1)
            act(3, 0)
            act(3, 1)

            # ---- PE fillers ----
            fc_pending = None
            if q < 3 and m >= 1:
                fc_pending = fc_chunk_mms(m - 1, q)
            # Heater matmuls into the (dead after its acts) f-chunk bank:
            # the PE HAM clock gate re-throttles to 1.2 GHz whenever the PE
            # idles through its activity window, and every matmul then runs
            # 2x slow.  Keeping the PE streaming through the nonlinear tail
            # pins the 2.4 GHz state, which is worth far more than these
            # cycles cost.  (Must precede emit_inject(t+1): its bank is
            # recycled as the (t+1, i) inject target.)
            heat_gt = gate_tiles[(t, 0)]
            xp_m = xp_tiles[t // 4]
            for hh in range(5):
                nc.tensor.matmul(
                    heat_gt[:], lhsT=eye4[0:32, :], rhs=xp_m[0:32, 0:512],
                    start=True, stop=True, skip_group_check=True,
                )
            # open next step's chunk groups
            if t + 1 < T:
                emit_inject(t + 1)

            # ---- c/h update, halves pipelined ----
            # c = sigmoid(f)*c + sigmoid(i)*tanh(g);  h = sigmoid(o)*tanh(c)
            # f*c on gpsimd (otherwise idle), everything else vector/scalar.
            fmul = work.tile([B, H], bf16, tag="fmul")
            ig = work.tile([B, H], bf16, tag="ig")
            tanhc = work.tile([B, H], bf16, tag="tanhc")
            h_t = work.tile([B, H], bf16, tag="h")
            # half 0 of f*c on vector (fast, heads the h0 chain); half 1 on
            # the otherwise-idle gpsimd, in parallel
            nc.vector.tensor_mul(fmul[:, 0:256], nl[:, 0:256], c_sb[:, 0:256])
            nc.gpsimd.tensor_mul(fmul[:, 256:512], nl[:, 256:512], c_sb[:, 256:512])
            nc.vector.tensor_mul(ig[:], nl[:, 512:1024], nl[:, 1024:1536])
            hp = h_psum.tile([128, 128], bf16, tag="hp")
            for half in (0, 1):
                hs = slice(256 * half, 256 * (half + 1))
                nc.vector.tensor_add(c_sb[:, hs], fmul[:, hs], ig[:, hs])
                nc.scalar.activation(tanhc[:, hs], c_sb[:, hs], AF.Tanh)
                nc.vector.tensor_mul(
                    h_t[:, hs], nl[:, 1536 + 256 * half : 1536 + 256 * (half + 1)],
                    tanhc[:, hs],
                )
                for k in (2 * half, 2 * half + 1):
                    nc.tensor.transpose(
                        hp[:, 32 * k : 32 * (k + 1)],
                        h_t[0:32, 128 * k : 128 * (k + 1)],
                        ident_b[0:32, 0:32],
                    )
                # fp8 copy (16*h.T) first: it gates the next step's DoubleRow
                # matmuls; the bf16 hsT copy (fc input) can lag.  half
                # doubles as the pair index P (k = 2*P + i).
                nc.vector.tensor_scalar_mul(
                    hsT8[:, half, :, 32 * t : 32 * (t + 1)],
                    hp[:, 64 * half : 64 * (half + 1)].rearrange(
                        "p (k b) -> p k b", k=2
                    ),
                    16.0,
                )
                nc.vector.tensor_copy(
                    hsT[:, 2 * half : 2 * half + 2, 32 * t : 32 * (t + 1)],
                    hp[:, 64 * half : 64 * (half + 1)].rearrange(
                        "p (k b) -> p k b", k=2
                    ),
                )

            # ---- non-critical work after the tail ----
            if fc_pending is not None:
                fc_chunk_finish(m - 1, q, fc_pending)
            if q == 0 and m + 2 <= 15:
                gather_xp(m + 2)

        for j in range(3):
            fps = fc_chunk_mms(15, j)
            fc_chunk_finish(15, j, fps)

    nc.compile()
    return nc


def _get_program():
    global _PROGRAM
    if _PROGRAM is None:
        _PROGRAM = _build_program()
    return _PROGRAM


# PyTorch LSTM gate order is [i, f, g, o]; we reorder rows to [f, i, g, o] so
# the f-sigmoid (head of the c-chain) is the first chunk to complete.
def _gate_perm():
    return np.concatenate(
        [np.arange(H, 2 * H), np.arange(0, H), np.arange(2 * H, 3 * H), np.arange(3 * H, 4 * H)]
    )


def _make_in_maps(features, captions, embed_table, W_ih, W_hh, b_ih, b_hh, fc_W, fc_b):
    import ml_dtypes

    bf16 = ml_dtypes.bfloat16
    f8e4 = ml_dtypes.float8_e4m3
    perm = _gate_perm()
    features = np.asarray(features, dtype=np.float32)
    cap = np.asarray(captions).astype(np.int32)                      # [B, T]
    embed = np.asarray(embed_table, dtype=np.float32)
    wihT_p = np.asarray(W_ih, dtype=np.float32)[perm].T              # [E, 4H]
    bsum = (np.asarray(b_ih, dtype=np.float32) + np.asarray(b_hh, dtype=np.float32))[perm]
    # Parameter-only precompute: gate-space embedding table and the t=0 row,
    # pre-scaled x1024 to match the fp8 recurrence PSUM scale.
    gtab = np.ascontiguousarray(((embed @ wihT_p + bsum) * 1024.0).astype(bf16))
    xp0 = np.ascontiguousarray(((features @ wihT_p + bsum) * 1024.0).astype(bf16))
    # recurrence weights in fp8 e4m3, pre-scaled x64 (h is 16x -> 1024x true)
    whh8 = np.ascontiguousarray(
        np.clip(np.asarray(W_hh, dtype=np.float32)[perm].T * 64.0, -240, 240).astype(f8e4)
    )
    fc_W = np.asarray(fc_W, dtype=np.float32)
    fc_b = np.asarray(fc_b, dtype=np.float32)

    # gather indices, t-major: xs row t*32+b = embed[captions[b, t-1]] for t>=1
    idx = np.zeros(TB, dtype=np.int32)
    idx[B:] = cap[:, : T - 1].T.reshape(-1)

    in_maps = []
    for c in range(N_CORES):
        sl = slice(VSL * c, VSL * (c + 1))
        fcwT = np.zeros((H, VPAD), dtype=bf16)
        fcwT[:, :VSL] = fc_W[sl].T.astype(bf16)
        fcbp = np.zeros(VPAD, dtype=bf16)
        fcbp[:VSL] = fc_b[sl].astype(bf16)
        in_maps.append(
            dict(
                xp0=xp0,
                idx=idx,
                gtab=gtab,
                whh8=whh8,
                fcwT=np.ascontiguousarray(fcwT),
                fcb=fcbp,
                onesv=np.ones(128, dtype=bf16),
            )
        )
    return in_maps


def _install_ntff_hook():
    """Wire up NTFF profiling: bass_utils wants antenv.axon_hooks, which this
    container lacks; build it from trn_agent_boot's ctypes hook."""
    import sys as _sys
    import types

    if "antenv.axon_hooks" in _sys.modules:
        return
    if "/root/.axon_site" not in _sys.path:
        _sys.path.insert(0, "/root/.axon_site")
    from trn_agent_boot.trn_boot import _ntff_profile_via_ctypes

    hook = _ntff_profile_via_ctypes("/opt/axon/libaxon_pjrt.so")
    mod = types.ModuleType("antenv.axon_hooks")
    mod._hook = hook
    mod.set_axon_ntff_profile_hook = lambda h: setattr(mod, "_hook", h)
    mod.get_axon_ntff_profile_hook = lambda: mod._hook
    _sys.modules["antenv.axon_hooks"] = mod

    # avoid S3 uploads from the trace path in this zero-egress container
    import concourse.bass_utils as bu

    bu.upload_artifacts = lambda tmpdir: f"local:{tmpdir}"


def run(inputs, trace=False, trace_cores=None):
    """Run on hardware; returns (full_output [B,T,V] f32, BassKernelResults)."""
    from concourse.bass_utils import run_bass_kernel_spmd

    if trace:
        _install_ntff_hook()

    nc = _get_program()
    in_maps = _make_in_maps(
        inputs["features"],
        inputs["captions"],
        inputs["embed_table"],
        inputs["W_ih"],
        inputs["W_hh"],
        inputs["b_ih"],
        inputs["b_hh"],
        inputs["fc_W"],
        inputs["fc_b"],
    )
    kwargs = {}
    if trace:
        import os
        import shutil

        shutil.rmtree("/tmp/bass_trace", ignore_errors=True)
        os.makedirs("/tmp/bass_trace", exist_ok=True)
        kwargs.update(trace=True, trace_cores=trace_cores or [0], tmpdir="/tmp/bass_trace")
    res = run_bass_kernel_spmd(nc, in_maps, core_ids=list(range(N_CORES)), **kwargs)
    full = np.concatenate(
        [np.asarray(r["out"]).astype(np.float32) for r in res.results], axis=2
    )
    return full, res


def kernel(**inputs) -> np.ndarray:
    out, _ = run(inputs, trace=False)
    return out


# revision 24
# speedup vs baseline: 1.0485x; 1.0485x over previous
"""Trainium2 Bass kernel for nn_DecoderRNN (LSTM decoder + big vocab projection).

Strategy (8 NeuronCores, SPMD):
  - LSTM recurrence (B=32, T=64, H=512) replicated on every core (its cost is
    batch-independent); output projection fc tensor-parallel over vocab:
    core c writes logits[:, :, 1250c:1250(c+1)], host concatenates.
  - The input-projection term of the gates is precomputed as a parameter-only
    product on the host:  G_table = embed @ W_ih.T + (b_ih + b_hh)  [V, 4H].
    On device, step t's xs-contribution is a single indirect-DMA gather of
    G_table rows by caption index (t=0 rows come from the host-computed
    features @ W_ih.T + b).  This replaces the on-device xs @ W_ih.T matmuls,
    the embedding gather + transposes, and all bias matmuls.
  - Gate pre-activations for step t accumulate in a [32, 512] PSUM bank per
    gate chunk: a stacked-identity "inject" matmul deposits the gathered
    xs-term (emitted at the end of step t-1, so it streams during the tail),
    then two fp8 DoubleRow matmuls add h(t-1) @ W_hh.T and close the group.
  - Scaling: G_table is host-prescaled x1024; hsT8 = 16x h, W_hh = 64x ->
    gate PSUM = 1024x true; activations descale by 1/1024.
  - Gate chunk order in SBUF columns is [f | i | g | o]: sigmoid(f) (which
    heads the c critical path via f*c) completes first.  The f*c multiply
    runs on the otherwise-idle gpsimd engine; the rest of the tail is on
    Vector.  fc drains emit bf16 and the host upcasts.

PSUM budget (8 banks): 5 gate banks (4 live + staging for next step's
injects) + 2 fc banks + 1 transpose bank.

kernel(**inputs) takes FULL unsharded inputs, returns FULL [32, 64, 10000] f32.
"""

import sys

sys.path.insert(0, "/opt/trn_rl_repo")

import numpy as np

N_CORES = 8
B, T = 32, 64
E, H, V = 512, 512, 10000
G4 = 4 * H            # 2048
TB = T * B            # 2048
VSL = V // N_CORES    # 1250 vocab rows per core
VPAD = 1280           # padded so fc N-chunks are 512/512/256 (all >=256)

_PROGRAM = None


def _build_program():
    import concourse.bass as bass
    import concourse.tile as tile
    from concourse import bacc, mybir
    from concourse.masks import make_identity
    from contextlib import ExitStack

    f32 = mybir.dt.float32
    bf16 = mybir.dt.bfloat16
    f8e4 = mybir.dt.float8e4
    i32 = mybir.dt.int32
    AF = mybir.ActivationFunctionType
    DR = mybir.MatmulPerfMode.DoubleRow

    nc = bacc.Bacc(
        "TRN2",
        target_bir_lowering=False,
        debug=False,
        num_devices=N_CORES,
    )

    xp0 = nc.dram_tensor("xp0", [B, G4], bf16, kind="ExternalInput").ap()
    idx = nc.dram_tensor("idx", [TB], i32, kind="ExternalInput").ap()
    gtab = nc.dram_tensor("gtab", [V, G4], bf16, kind="ExternalInput").ap()
    whh8 = nc.dram_tensor("whh8", [H, G4], f8e4, kind="ExternalInput").ap()
    fcwT = nc.dram_tensor("fcwT", [H, VPAD], bf16, kind="ExternalInput").ap()
    fcb = nc.dram_tensor("fcb", [VPAD], bf16, kind="ExternalInput").ap()
    onesv = nc.dram_tensor("onesv", [128], bf16, kind="ExternalInput").ap()
    out = nc.dram_tensor("out", [B, T, VSL], bf16, kind="ExternalOutput").ap()
    # Output viewed as [t, b, v]: a 128-row t-major tb tile = 4 t planes.
    out_r = out.rearrange("b t v -> t b v")

    with tile.TileContext(nc) as tc, ExitStack() as ctx:
        # ---------------- persistent state ----------------
        state = ctx.enter_context(tc.tile_pool(name="state", bufs=1))
        # h.T history: block t holds h(t).T (written at the end of step t).
        # Layout [p, k, 32*t + b] = h(t)[b, 128*k + p]
        hsT = state.tile([128, 4, 32 * T], bf16, tag="hsT")
        # fp8 16x copy for the DoubleRow recurrence matmuls: pair layout
        # [p, P, i, .] = contraction row 128*(2P+i)+p.
        hsT8 = state.tile([128, 2, 2, 32 * T], f8e4, tag="hsT8")
        whh8_sb = state.tile([128, 2, 2, G4], f8e4, tag="whh8")
        fcwT_sb = state.tile([128, 4, VPAD], bf16, tag="fcwT")
        fcb_sb = state.tile([1, VPAD], bf16, tag="fcb")
        fcb128 = state.tile([128, VPAD], f32, tag="fcb128")
        c_sb = state.tile([B, H], bf16, tag="c")
        ident_b = state.tile([128, 128], bf16, tag="ident_b")
        ones = state.tile([1, 128], bf16, tag="ones")
        eye4 = state.tile([128, 32], bf16, tag="eye4")   # I32 stacked 4x
        idx_sb = state.tile([128, 16], i32, tag="idx")

        make_identity(nc, ident_b[:])
        nc.vector.memset(c_sb[:], 0.0)

        nc.sync.dma_start(fcb_sb[:], fcb[None, :])
        nc.sync.dma_start(ones[:], onesv[None, :])
        for qq in range(4):
            nc.sync.dma_start(eye4[32 * qq : 32 * (qq + 1), :], ident_b[0:32, 0:32])

        # ---------------- PSUM pools ----------------
        g_psum = ctx.enter_context(tc.tile_pool(name="g_ps", bufs=5, space="PSUM"))
        fc_psum = ctx.enter_context(tc.tile_pool(name="fc_ps", bufs=2, space="PSUM"))
        h_psum = ctx.enter_context(tc.tile_pool(name="h_ps", bufs=1, space="PSUM"))

        # gathered G_table rows (xs gate contribution), [tb-tile, 4H] ring
        xp_ring = ctx.enter_context(tc.tile_pool(name="xp_ring", bufs=3))
        xp_tiles = {}
        gate_tiles = {}

        def gather_xp(m):
            """xp tile m = G_table rows for tb 128m..128m+128 (gate space)."""
            xp_m = xp_ring.tile([128, G4], bf16, tag="xp_m")
            xp_tiles[m] = xp_m
            nc.gpsimd.indirect_dma_start(
                out=xp_m[:],
                out_offset=None,
                in_=gtab[:, :],
                in_offset=bass.IndirectOffsetOnAxis(
                    ap=idx_sb[:, m : m + 1], axis=0
                ),
            )
            if m == 0:
                # t=0 rows: host-computed features @ W_ih.T + b
                nc.gpsimd.dma_start(xp_m[0:32, :], xp0[:, :])

        def emit_inject(t):
            """Open step t's four chunk groups with the gathered xs term."""
            q = t % 4
            xp_m = xp_tiles[t // 4]
            for cch in range(4):
                sl = slice(512 * cch, 512 * (cch + 1))
                gt = g_psum.tile([B, 512], f32, tag="g")
                gate_tiles[(t, cch)] = gt
                nc.tensor.matmul(
                    gt[:],
                    lhsT=eye4[32 * q : 32 * (q + 1), :],
                    rhs=xp_m[32 * q : 32 * (q + 1), sl],
                    start=True,
                    stop=(t == 0),
                    tile_position=(32 * q, 0),
                    skip_group_check=True,
                )

        # ---------------- prologue ----------------
        with ExitStack() as pro:
            nc.sync.dma_start(idx_sb[:], idx.rearrange("(m p) -> p m", p=128))
            nc.sync.dma_start(
                whh8_sb[:], whh8.rearrange("(P i p) g -> p P i g", P=2, i=2)
            )
            nc.sync.dma_start(fcwT_sb[:], fcwT.rearrange("(k p) v -> p k v", p=128))
            # fcb128 = broadcast(fc_b) via rank-1 matmuls into the fc bank
            for c0, csz in ((0, 512), (512, 512), (1024, 256)):
                bp = fc_psum.tile([128, 512], f32, tag="fc")
                nc.tensor.matmul(bp[:, 0:csz], lhsT=ones[0:1, :],
                                 rhs=fcb_sb[0:1, c0 : c0 + csz], start=True, stop=True)
                nc.vector.tensor_copy(fcb128[:, c0 : c0 + csz], bp[:, 0:csz])

            gather_xp(0)
            gather_xp(1)
            emit_inject(0)

        # ---------------- main recurrence + interleaved fc ----------------
        work = ctx.enter_context(tc.tile_pool(name="work", bufs=3))
        lg_pool = ctx.enter_context(tc.tile_pool(name="lg", bufs=2))

        FC_CHUNKS = ((0, 512), (512, 512), (1024, 256))
        lg_tiles = {}

        def fc_chunk_mms(m, j):
            """PE part of fc chunk j for tb tile m (fills PE bubbles)."""
            if j == 0:
                lg_new = lg_pool.tile([128, VPAD], bf16, tag="lg")
                lg_tiles[m] = lg_new
            c0, csz = FC_CHUNKS[j]
            fps = fc_psum.tile([128, 512], f32, tag="fc")
            for k in range(4):
                nc.tensor.matmul(
                    fps[:, 0:csz],
                    lhsT=hsT[:, k, 128 * m : 128 * (m + 1)],
                    rhs=fcwT_sb[:, k, c0 : c0 + csz],
                    start=(k == 0),
                    stop=(k == 3),
                )
            return fps

        def fc_chunk_finish(m, j, fps):
            c0, csz = FC_CHUNKS[j]
            nc.vector.tensor_add(
                lg_tiles[m][:, c0 : c0 + csz], fps[:, 0:csz], fcb128[:, c0 : c0 + csz]
            )
            if j == 2:
                # DRAM side is [4 t, 32 b, 1250 v]; SBUF side [128, 1250]
                # pairs element-stream-wise (partition p = 32*t_local + b).
                nc.sync.dma_start(
                    out_r[4 * m : 4 * (m + 1), :, :], lg_tiles[m][:, 0:VSL]
                )

        # gate chunk order in SBUF columns (host permutes): 0=f 1=i 2=g 3=o
        for t in range(T):
            q = t % 4
            m = t // 4
            nl = work.tile([B, G4], bf16, tag="nl")

            # ---- close the chunk groups with h(t-1) @ W_hh.T; act ASAP ----
            # f-chunk runs in 256-col quarters so sigmoid(f) half 0 (which
            # heads the c critical path) starts after just two matmuls.
            for cch in range(4):
                gt = gate_tiles[(t, cch)]
                if t > 0:
                    nhalves = (0, 1) if cch == 0 else (None,)
                    for nh in nhalves:
                        ns = slice(0, 512) if nh is None else slice(256 * nh, 256 * (nh + 1))
                        for P in (0, 1):
                            nc.tensor.matmul(
                                gt[:, ns],
                                lhsT=hsT8[:, P, :, 32 * (t - 1) : 32 * t],
                                rhs=whh8_sb[:, P, :, 512 * cch + ns.start : 512 * cch + ns.stop],
                                start=False,
                                stop=(P == 1),
                                perf_mode=DR,
                                skip_group_check=True,
                            )
            # Scalar act queue order (f0, i, g, f1, o0, o1): f0 heads the
            # f*c chain; i and g feed sigmoid(i)*tanh(g) next; f1 (whose
            # consumer f*c-half1 runs on gpsimd, off the critical path) is
            # deferred behind g so the ig-product path isn't delayed.
            def act(cch, ah=None):
                g_tile = gate_tiles[(t, cch)]
                if ah is None:
                    nc.scalar.activation(
                        nl[:, 512 * cch : 512 * (cch + 1)], g_tile[:],
                        AF.Tanh if cch == 2 else AF.Sigmoid, scale=1.0 / 1024.0,
                    )
                else:
                    nc.scalar.activation(
                        nl[:, 512 * cch + 256 * ah : 512 * cch + 256 * (ah + 1)],
                        g_tile[:, 256 * ah : 256 * (ah + 1)],
                        AF.Sigmoid, scale=1.0 / 1024.0,
                    )

            act(0, 0)
            act(1)
            act(2)
            act(0, 1)
            act(3, 0)
            act(3, 1)

            # open next step's chunk groups
            if t + 1 < T:
                emit_inject(t + 1)

            # ---- c/h update, halves pipelined ----
            # c = sigmoid(f)*c + sigmoid(i)*tanh(g);  h = sigmoid(o)*tanh(c)
            # f*c on gpsimd (otherwise idle), everything else vector/scalar.
            fmul = work.tile([B, H], bf16, tag="fmul")
            ig = work.tile([B, H], bf16, tag="ig")
            tanhc = work.tile([B, H], bf16, tag="tanhc")
            h_t = work.tile([B, H], bf16, tag="h")
            # half 0 of f*c on vector (fast, heads the h0 chain); half 1 on
            # the otherwise-idle gpsimd, in parallel
            nc.vector.tensor_mul(fmul[:, 0:256], nl[:, 0:256], c_sb[:, 0:256])
            nc.gpsimd.tensor_mul(fmul[:, 256:512], nl[:, 256:512], c_sb[:, 256:512])
            nc.vector.tensor_mul(ig[:], nl[:, 512:1024], nl[:, 1024:1536])
            hp = h_psum.tile([128, 128], bf16, tag="hp")
            for half in (0, 1):
                hs = slice(256 * half, 256 * (half + 1))
                nc.vector.tensor_add(c_sb[:, hs], fmul[:, hs], ig[:, hs])
                nc.scalar.activation(tanhc[:, hs], c_sb[:, hs], AF.Tanh)
                nc.vector.tensor_mul(
                    h_t[:, hs], nl[:, 1536 + 256 * half : 1536 + 256 * (half + 1)],
                    tanhc[:, hs],
                )
                for k in (2 * half, 2 * half + 1):
                    nc.tensor.transpose(
                        hp[:, 32 * k : 32 * (k + 1)],
                        h_t[0:32, 128 * k : 128 * (k + 1)],
                        ident_b[0:32, 0:32],
                    )
                # fp8 copy (16*h.T) first: it gates the next step's DoubleRow
                # matmuls; the bf16 hsT copy (fc input) can lag.  half
                # doubles as the pair index P (k = 2*P + i).
                nc.vector.tensor_scalar_mul(
                    hsT8[:, half, :, 32 * t : 32 * (t + 1)],
                    hp[:, 64 * half : 64 * (half + 1)].rearrange(
                        "p (k b) -> p k b", k=2
                    ),
                    16.0,
                )
                nc.vector.tensor_copy(
                    hsT[:, 2 * half : 2 * half + 2, 32 * t : 32 * (t + 1)],
                    hp[:, 64 * half : 64 * (half + 1)].rearrange(
                        "p (k b) -> p k b", k=2
                    ),
                )

            # ---- non-critical work after the tail ----
            if q == 0 and m + 2 <= 15:
                gather_xp(m + 2)

        # ---- fc epilogue: one contiguous matmul burst over all blocks ----
        # A solid stream of back-to-back matmuls holds the PE HAM clock gate
        # at 2.4 GHz, unlike fc chunks sprinkled between recurrence steps.
        for mb in range(16):
            for j in range(3):
                fps = fc_chunk_mms(mb, j)
                fc_chunk_finish(mb, j, fps)

    nc.compile()
    return nc


def _get_program():
    global _PROGRAM
    if _PROGRAM is None:
        _PROGRAM = _build_program()
    return _PROGRAM


# PyTorch LSTM gate order is [i, f, g, o]; we reorder rows to [f, i, g, o] so
# the f-sigmoid (head of the c-chain) is the first chunk to complete.
def _gate_perm():
    return np.concatenate(
        [np.arange(H, 2 * H), np.arange(0, H), np.arange(2 * H, 3 * H), np.arange(3 * H, 4 * H)]
    )


def _make_in_maps(features, captions, embed_table, W_ih, W_hh, b_ih, b_hh, fc_W, fc_b):
    import ml_dtypes

    bf16 = ml_dtypes.bfloat16
    f8e4 = ml_dtypes.float8_e4m3
    perm = _gate_perm()
    features = np.asarray(features, dtype=np.float32)
    cap = np.asarray(captions).astype(np.int32)                      # [B, T]
    embed = np.asarray(embed_table, dtype=np.float32)
    wihT_p = np.asarray(W_ih, dtype=np.float32)[perm].T              # [E, 4H]
    bsum = (np.asarray(b_ih, dtype=np.float32) + np.asarray(b_hh, dtype=np.float32))[perm]
    # Parameter-only precompute: gate-space embedding table and the t=0 row,
    # pre-scaled x1024 to match the fp8 recurrence PSUM scale.
    gtab = np.ascontiguousarray(((embed @ wihT_p + bsum) * 1024.0).astype(bf16))
    xp0 = np.ascontiguousarray(((features @ wihT_p + bsum) * 1024.0).astype(bf16))
    # recurrence weights in fp8 e4m3, pre-scaled x64 (h is 16x -> 1024x true)
    whh8 = np.ascontiguousarray(
        np.clip(np.asarray(W_hh, dtype=np.float32)[perm].T * 64.0, -240, 240).astype(f8e4)
    )
    fc_W = np.asarray(fc_W, dtype=np.float32)
    fc_b = np.asarray(fc_b, dtype=np.float32)

    # gather indices, t-major: xs row t*32+b = embed[captions[b, t-1]] for t>=1
    idx = np.zeros(TB, dtype=np.int32)
    idx[B:] = cap[:, : T - 1].T.reshape(-1)

    in_maps = []
    for c in range(N_CORES):
        sl = slice(VSL * c, VSL * (c + 1))
        fcwT = np.zeros((H, VPAD), dtype=bf16)
        fcwT[:, :VSL] = fc_W[sl].T.astype(bf16)
        fcbp = np.zeros(VPAD, dtype=bf16)
        fcbp[:VSL] = fc_b[sl].astype(bf16)
        in_maps.append(
            dict(
                xp0=xp0,
                idx=idx,
                gtab=gtab,
                whh8=whh8,
                fcwT=np.ascontiguousarray(fcwT),
                fcb=fcbp,
                onesv=np.ones(128, dtype=bf16),
            )
        )
    return in_maps


def _install_ntff_hook():
    """Wire up NTFF profiling: bass_utils wants antenv.axon_hooks, which this
    container lacks; build it from trn_agent_boot's ctypes hook."""
    import sys as _sys
    import types

    if "antenv.axon_hooks" in _sys.modules:
        return
    if "/root/.axon_site" not in _sys.path:
        _sys.path.insert(0, "/root/.axon_site")
    from trn_agent_boot.trn_boot import _ntff_profile_via_ctypes

    hook = _ntff_profile_via_ctypes("/opt/axon/libaxon_pjrt.so")
    mod = types.ModuleType("antenv.axon_hooks")
    mod._hook = hook
    mod.set_axon_ntff_profile_hook = lambda h: setattr(mod, "_hook", h)
    mod.get_axon_ntff_profile_hook = lambda: mod._hook
    _sys.modules["antenv.axon_hooks"] = mod

    # avoid S3 uploads from the trace path in this zero-egress container
    import concourse.bass_utils as bu

    bu.upload_artifacts = lambda tmpdir: f"local:{tmpdir}"


def run(inputs, trace=False, trace_cores=None):
    """Run on hardware; returns (full_output [B,T,V] f32, BassKernelResults)."""
    from concourse.bass_utils import run_bass_kernel_spmd

    if trace:
        _install_ntff_hook()

    nc = _get_program()
    in_maps = _make_in_maps(
        inputs["features"],
        inputs["captions"],
        inputs["embed_table"],
        inputs["W_ih"],
        inputs["W_hh"],
        inputs["b_ih"],
        inputs["b_hh"],
        inputs["fc_W"],
        inputs["fc_b"],
    )
    kwargs = {}
    if trace:
        import os
        import shutil

        shutil.rmtree("/tmp/bass_trace", ignore_errors=True)
        os.makedirs("/tmp/bass_trace", exist_ok=True)
        kwargs.update(trace=True, trace_cores=trace_cores or [0], tmpdir="/tmp/bass_trace")
    res = run_bass_kernel_spmd(nc, in_maps, core_ids=list(range(N_CORES)), **kwargs)
    full = np.concatenate(
        [np.asarray(r["out"]).astype(np.float32) for r in res.results], axis=2
    )
    return full, res


def kernel(**inputs) -> np.ndarray:
    out, _ = run(inputs, trace=False)
    return out
